# revision 1
# baseline (speedup 1.0000x reference)
"""GATConv (PyG defaults: add_self_loops, concat=False/head-mean) on 8 Trainium2 cores.

Strategy: edges are bucketed by DESTINATION node. Core k owns the NPC-node
slice [k*NPC, (k+1)*NPC) and every edge whose dst lands there, so the
segment softmax and the message aggregation are entirely core-local — no
device collectives. The host concatenates the 8 output slices.

Device program (SPMD-identical across cores; all data-dependent structure
lives in index arrays; per-core addressing is impossible, so per-core data
is always reached through host-supplied index arrays):

  Phase 1 (replicated): h = x @ [W | W.att_src | W.att_dst] on the PE ->
      fp16 h-table [NPAD, 640] in HBM. Row n = [h(n) 512 | a_s(n) 4xf32 |
      a_d(n) 4xf32 | pad], 1280 B (dma_gather needs 256B-divisible rows).

  Phase 2:
    - build a core-LOCAL a_d table [NPC_pad, 128] f16 (256 B rows): one
      single-index indirect gather per 128-node block (block node ids come
      from a host index array), so the later per-edge a_d dma_gather can
      use local dst indices (< NPC, fits int16).
    - edges sorted by dst, split per (block, src-half) so every gather
      group reads one half of the h-table with rebased int16 indices
      (dma_gather indices are signed int16; N=50000 doesn't fit, halves do)
    - per 128-edge tile: e = exp(leaky_relu(a_s+a_d))/16 (= max of two
      exps; no segment max needed - logits are O(10) and softmax is
      shift-invariant); one-hot dst-selection S01 = is_equal(dloc, iota);
      PE: psum_out += S01^T @ (e (x) h_src), psum_den += S01^T @ e
    - per block: out = sum_h psum_out[:,h] / (H*den[:,h]) -> HBM
"""

import math
import sys

import numpy as np

if "/opt/trn_rl_repo" not in sys.path:
    sys.path.insert(0, "/opt/trn_rl_repo")

P = 128
SLOPE = 0.2
LN16 = float(np.log(16.0))
HROW = 640          # padded h-table row (f16 elems): 512 h + 8 a_s + 8 a_d + pad
ADROW = 128         # local a_d table row (f16 elems) = 256 B


class Cfg:
    def __init__(self, N=50000, E=800000, DIN=128, DOUT=128, H=4, ncores=8):
        self.N, self.E, self.DIN, self.DOUT, self.H = N, E, DIN, DOUT, H
        self.NCORES = ncores
        self.NPC = N // ncores                 # nodes per core
        self.NBLK = math.ceil(self.NPC / P)    # dst blocks per core
        self.LAST_ROWS = self.NPC - (self.NBLK - 1) * P
        self.NPAD = math.ceil(N / P) * P       # padded node count
        self.NTILE_N = self.NPAD // P          # node tiles in phase 1
        self.WH = H * DOUT                     # h width = 512
        self.NSPLIT = self.NPAD // 2           # h-table half split row
        self.NPC_PAD = self.NBLK * P           # local a_d table rows
        assert DIN == P and self.WH == 512 and H * DOUT == 512
        assert self.NSPLIT < 32768 and self.NPAD - self.NSPLIT < 32768
        assert self.NPC_PAD < 32768


DEFAULT_CFG = Cfg()


def _build_program(cfg: Cfg, t_half: int):
    """nt = NBLK * 2 * t_half edge tiles per core, all data via index arrays."""
    from contextlib import ExitStack

    import concourse.bacc as bacc
    import concourse.bass as bass
    import concourse.mybir as mybir
    import concourse.tile as tile

    f16 = mybir.dt.float16
    f32 = mybir.dt.float32
    i32 = mybir.dt.int32
    i16 = mybir.dt.int16
    AF = mybir.ActivationFunctionType
    WH, H, DOUT = cfg.WH, cfg.H, cfg.DOUT
    nt = cfg.NBLK * 2 * t_half
    NIH = t_half * P          # idxs per h-gather group (one (block, half))
    NIA = 2 * t_half * P      # idxs per a_d-gather group (one block)

    nc = bacc.Bacc(
        "TRN2",
        target_bir_lowering=False,
        debug=False,
        enable_asserts=False,
        num_devices=cfg.NCORES,
    )

    xT = nc.dram_tensor("xT", [cfg.NTILE_N, P, P], f16, kind="ExternalInput").ap()
    wext = nc.dram_tensor("wext", [P, WH + 2 * H], f16, kind="ExternalInput").ap()
    iota_in = nc.dram_tensor("iota", [P, P], f16, kind="ExternalInput").ap()
    hidx_in = nc.dram_tensor("hidx", [P, nt * 8], i16, kind="ExternalInput").ap()
    adidx_in = nc.dram_tensor("adidx", [P, nt * 8], i16, kind="ExternalInput").ap()
    dlocv_in = nc.dram_tensor("dlocv", [P, nt], f16, kind="ExternalInput").ap()
    bidx_in = nc.dram_tensor("bidx", [P, cfg.NBLK], i32, kind="ExternalInput").ap()
    out = nc.dram_tensor("out", [cfg.NPC, DOUT], f32, kind="ExternalOutput").ap()
    htab = nc.dram_tensor("htab", [cfg.NPAD, HROW], f16, kind="Internal").ap()
    adloc = nc.dram_tensor("adloc", [cfg.NPC_PAD, ADROW], f16, kind="Internal").ap()

    with tile.TileContext(nc) as tc:
        with ExitStack() as ctx:
            cpool = ctx.enter_context(tc.tile_pool(name="const", bufs=1))
            wext_sb = cpool.tile([P, WH + 2 * H], f16)
            nc.sync.dma_start(wext_sb[:], wext[:, :])

            # ---------------- Phase 1: h table ----------------
            with (
                tc.tile_pool(name="p1x", bufs=4) as p1x,
                tc.tile_pool(name="p1h", bufs=4) as p1h,
                tc.tile_pool(name="p1ph", bufs=4, space="PSUM") as p1ph,
                tc.tile_pool(name="p1ps", bufs=4, space="PSUM") as p1ps,
            ):
                for t in range(cfg.NTILE_N):
                    xt = p1x.tile([P, P], f16)
                    nc.sync.dma_start(xt[:], xT[t, :, :])
                    ph = p1ph.tile([P, WH], f32, space="PSUM")
                    ps = p1ps.tile([P, 2 * H], f32, space="PSUM")
                    nc.tensor.matmul(
                        ph[:], lhsT=xt[:], rhs=wext_sb[:, 0:WH], start=True, stop=True
                    )
                    nc.tensor.matmul(
                        ps[:], lhsT=xt[:], rhs=wext_sb[:, WH:], start=True, stop=True
                    )
                    hsb = p1h.tile([P, HROW], f16)
                    c0 = (WH * 3) // 4
                    nc.vector.tensor_copy(hsb[:, 0:c0], ph[:, 0:c0])
                    nc.scalar.activation(hsb[:, c0:WH], ph[:, c0:WH], AF.Copy)
                    # a_s/a_d as raw fp32 bits in f16 slots [512:528)
                    nc.vector.tensor_copy(
                        hsb[:, WH : WH + 4 * H].bitcast(f32), ps[:]
                    )
                    nc.sync.dma_start(
                        htab[t * P : (t + 1) * P, 0 : WH + 4 * H],
                        hsb[:, 0 : WH + 4 * H],
                    )

            tc.strict_bb_all_engine_barrier()

            # ---------------- Phase 2a: local a_d table ----------------
            bidx_sb = cpool.tile([P, cfg.NBLK], i32)
            nc.sync.dma_start(bidx_sb[:], bidx_in[:, :])
            with tc.tile_pool(name="adst", bufs=4) as adst_pool:
                for b in range(cfg.NBLK):
                    ssb = adst_pool.tile([P, 2 * H], f16)
                    nc.gpsimd.indirect_dma_start(
                        out=ssb[:],
                        out_offset=None,
                        in_=htab[:, :],
                        in_offset=bass.IndirectOffsetOnAxis(
                            ap=bidx_sb[:, b : b + 1], axis=0
                        ),
                        element_offset=WH + 2 * H,  # a_d fp32 slots
                    )
                    nc.sync.dma_start(
                        adloc[b * P : (b + 1) * P, 0 : 2 * H], ssb[:]
                    )

            tc.strict_bb_all_engine_barrier()

            # ---------------- Phase 2b: edge processing ----------------
            iota_sb = cpool.tile([P, P], f16)
            nc.sync.dma_start(iota_sb[:], iota_in[:, :])
            bln16 = cpool.tile([P, 1], f32)
            nc.vector.memset(bln16[:], -LN16)
            hidx = cpool.tile([P, nt * 8], i16)
            nc.sync.dma_start(hidx[:], hidx_in[:, :])
            adidx = cpool.tile([P, nt * 8], i16)
            nc.sync.dma_start(adidx[:], adidx_in[:, :])
            dlocv = cpool.tile([P, nt], f16)
            nc.sync.dma_start(dlocv[:], dlocv_in[:, :])

            htabA = htab[0 : cfg.NSPLIT, :]
            htabB = htab[cfg.NSPLIT :, :]

            with (
                tc.tile_pool(name="gh", bufs=3) as gh_pool,
                tc.tile_pool(name="gad", bufs=3) as gad_pool,
                tc.tile_pool(name="s01", bufs=4) as s01_pool,
                tc.tile_pool(name="hwp", bufs=4) as hw_pool,
                tc.tile_pool(name="sm", bufs=8) as sm_pool,
                tc.tile_pool(name="evp", bufs=8) as ev_pool,
                tc.tile_pool(name="fin", bufs=3) as fin_pool,
                tc.tile_pool(name="fin2", bufs=3) as fin2_pool,
                tc.tile_pool(name="pso", bufs=2, space="PSUM") as pso_pool,
                tc.tile_pool(name="psd", bufs=2, space="PSUM") as psd_pool,
            ):
                for b in range(cfg.NBLK):
                    # one a_d gather per block (both halves' tiles)
                    gad = gad_pool.tile([P, 2 * t_half * ADROW], f16)
                    nc.gpsimd.dma_gather(
                        out_ap=gad[:].rearrange("p (k e) -> p k e", e=ADROW),
                        in_ap=adloc[:, :],
                        idxs_ap=adidx[:, b * NIA // 16 : (b + 1) * NIA // 16],
                        num_idxs=NIA,
                        num_idxs_reg=NIA,
                        elem_size=ADROW,
                        single_packet=False,
                    )
                    psum_out = pso_pool.tile([P, WH], f32, space="PSUM")
                    psum_den = psd_pool.tile([P, H], f32, space="PSUM")
                    for hh in range(2):
                        g = b * 2 + hh  # h-gather group id
                        gh = gh_pool.tile([P, t_half * HROW], f16)
                        nc.gpsimd.dma_gather(
                            out_ap=gh[:].rearrange("p (k e) -> p k e", e=HROW),
                            in_ap=htabA if hh == 0 else htabB,
                            idxs_ap=hidx[:, g * NIH // 16 : (g + 1) * NIH // 16],
                            num_idxs=NIH,
                            num_idxs_reg=NIH,
                            elem_size=HROW,
                            single_packet=False,
                        )
                        for s in range(t_half):
                            t = (b * 2 + hh) * t_half + s
                            tt = hh * t_half + s  # slot in block order
                            gj = gh[:, s * HROW : s * HROW + WH]
                            asj = gh[
                                :, s * HROW + WH : s * HROW + WH + 2 * H
                            ].bitcast(f32)
                            adj = gad[
                                :, tt * ADROW : tt * ADROW + 2 * H
                            ].bitcast(f32)
                            # e = exp(lrelu(z))/16 = max(exp(z-ln16), exp(.2z-ln16))
                            z = sm_pool.tile([P, H], f32, tag="z")
                            nc.vector.tensor_add(z[:], asj, adj)
                            e1 = sm_pool.tile([P, H], f32, tag="e1")
                            nc.scalar.activation(e1[:], z[:], AF.Exp, bias=bln16[:])
                            e2 = sm_pool.tile([P, H], f32, tag="e2")
                            nc.scalar.activation(
                                e2[:], z[:], AF.Exp, bias=bln16[:], scale=SLOPE
                            )
                            ev = ev_pool.tile([P, H], f32)
                            nc.vector.tensor_tensor(
                                out=ev[:], in0=e1[:], in1=e2[:],
                                op=mybir.AluOpType.max,
                            )
                            ev16 = ev_pool.tile([P, H], f16, tag="ev16")
                            nc.vector.tensor_copy(ev16[:], ev[:])
                            s01 = s01_pool.tile([P, P], f16)
                            nc.vector.tensor_tensor(
                                out=s01[:],
                                in0=dlocv[:, t : t + 1].to_broadcast([P, P]),
                                in1=iota_sb[:],
                                op=mybir.AluOpType.is_equal,
                            )
                            hw = hw_pool.tile([P, WH], f16)
                            for hd in range(H):
                                nc.vector.tensor_scalar_mul(
                                    hw[:, hd * DOUT : (hd + 1) * DOUT],
                                    gj[:, hd * DOUT : (hd + 1) * DOUT],
                                    ev[:, hd : hd + 1],
                                )
                            first = tt == 0
                            last = tt == 2 * t_half - 1
                            nc.tensor.matmul(
                                psum_out[:], lhsT=s01[:], rhs=hw[:],
                                start=first, stop=last,
                            )
                            nc.tensor.matmul(
                                psum_den[:], lhsT=s01[:], rhs=ev16[:],
                                start=first, stop=last,
                            )
                    # normalize + head mean
                    den4 = sm_pool.tile([P, H], f32, tag="den4")
                    nc.vector.tensor_scalar_mul(den4[:], psum_den[:], float(H))
                    rec = sm_pool.tile([P, H], f32, tag="rec")
                    nc.vector.reciprocal(rec[:], den4[:])
                    acc = fin_pool.tile([P, DOUT], f32)
                    nc.vector.tensor_scalar_mul(
                        acc[:], psum_out[:, 0:DOUT], rec[:, 0:1]
                    )
                    for hd in range(1, H):
                        tmp = fin2_pool.tile([P, DOUT], f32)
                        nc.vector.tensor_scalar_mul(
                            tmp[:],
                            psum_out[:, hd * DOUT : (hd + 1) * DOUT],
                            rec[:, hd : hd + 1],
                        )
                        nc.vector.tensor_add(acc[:], acc[:], tmp[:])
                    rows = cfg.LAST_ROWS if b == cfg.NBLK - 1 else P
                    nc.sync.dma_start(
                        out[b * P : b * P + rows, :], acc[:rows, :]
                    )

    nc.compile()
    return nc


def _wrap16(idx_flat, ni_per_group):
    """[G*NI] edge-position-ordered idx -> [128, G*NI/16] wrapped-16 layout,
    replicated across the 8 16-partition groups."""
    g = idx_flat.reshape(-1, ni_per_group)
    ng = g.shape[0]
    w = np.zeros((16, ng, ni_per_group // 16), np.int16)
    for p in range(16):
        w[p] = g[:, p::16]
    w = w.reshape(16, ng * (ni_per_group // 16))
    return np.tile(w, (8, 1))


def _prep(cfg: Cfg, x, edge_index, W, att_src, att_dst):
    """Host-side sharding/preprocessing -> (per-core in_maps, t_half)."""
    f16 = np.float16
    N, H, DIN, DOUT = cfg.N, cfg.H, cfg.DIN, cfg.DOUT
    src = np.concatenate([np.asarray(edge_index[0]), np.arange(N)]).astype(np.int64)
    dst = np.concatenate([np.asarray(edge_index[1]), np.arange(N)]).astype(np.int64)

    # sort edges by (core, block, src-half): key = dst-block * 2 + half
    core = dst // cfg.NPC
    ln = dst - core * cfg.NPC
    half = (src >= cfg.NSPLIT).astype(np.int64)
    key = (core * cfg.NBLK + ln // P) * 2 + half
    order = np.argsort(key, kind="stable")
    src_s = src[order].astype(np.int32)
    ln_s = ln[order].astype(np.int32)
    key_s = key[order]

    nseg = cfg.NCORES * cfg.NBLK * 2
    counts = np.bincount(key_s, minlength=nseg)
    t_half = int(max(1, ((counts + P - 1) // P).max()))
    seg_len = t_half * P
    nt = cfg.NBLK * 2 * t_half

    starts = np.concatenate([[0], np.cumsum(counts)])
    # padded per-core flat arrays in (block, half, slot, partition) order
    hsrc = np.zeros((cfg.NCORES, nt * P), np.int32)   # half-rebased src
    dloc = np.full((cfg.NCORES, nt * P), -1.0, f16)
    dl_i = np.zeros((cfg.NCORES, nt * P), np.int32)   # local dst (pad -> 0)
    for c in range(cfg.NCORES):
        for b in range(cfg.NBLK):
            for hh in range(2):
                seg = (c * cfg.NBLK + b) * 2 + hh
                s, e = starts[seg], starts[seg + 1]
                cnt = e - s
                o = ((b * 2 + hh) * t_half) * P
                sr = src_s[s:e] - (cfg.NSPLIT if hh else 0)
                hsrc[c, o : o + cnt] = sr
                dl_i[c, o : o + cnt] = ln_s[s:e]
                dloc[c, o : o + cnt] = (ln_s[s:e] - b * P).astype(f16)

    xpad = np.zeros((cfg.NPAD, DIN), np.float32)
    xpad[:N] = np.asarray(x)
    xT = np.ascontiguousarray(
        xpad.T.reshape(DIN, cfg.NTILE_N, P).transpose(1, 0, 2)
    ).astype(f16)
    Wn = np.asarray(W, dtype=np.float32)
    ws = np.einsum("khc,hc->kh", Wn.reshape(DIN, H, DOUT), np.asarray(att_src, np.float32))
    wd = np.einsum("khc,hc->kh", Wn.reshape(DIN, H, DOUT), np.asarray(att_dst, np.float32))
    wext = np.concatenate([Wn, ws, wd], axis=1).astype(f16)
    iota = np.broadcast_to(np.arange(P, dtype=f16), (P, P)).copy()
    # block node ids (global), clamped to the core's range
    in_maps = []
    for c in range(cfg.NCORES):
        bid = (
            c * cfg.NPC
            + np.minimum(
                np.arange(cfg.NBLK)[None, :] * P + np.arange(P)[:, None],
                cfg.NPC - 1,
            )
        ).astype(np.int32)
        in_maps.append(
            {
                "xT": xT,
                "wext": wext,
                "iota": iota,
                "hidx": _wrap16(hsrc[c], t_half * P),
                "adidx": _wrap16(dl_i[c], 2 * t_half * P),
                "dlocv": np.ascontiguousarray(dloc[c].reshape(nt, P).T),
                "bidx": bid,
            }
        )
    return in_maps, t_half


def run(cfg: Cfg, x, edge_index, W, att_src, att_dst, trace=False, sim=False):
    in_maps, t_half = _prep(cfg, x, edge_index, W, att_src, att_dst)
    nc = _build_program(cfg, t_half)
    if sim:
        from concourse.bass_interp import CoreSim

        outs = []
        for c in range(cfg.NCORES):
            s = CoreSim(nc, trace=False, require_finite=False, require_nnan=False)
            for k, v in in_maps[c].items():
                s.tensor(k)[:] = v
            s.simulate(check_with_hw=False)
            outs.append(np.array(s.tensor("out")))
        return np.concatenate(outs, axis=0), None
    from concourse.bass_utils import run_bass_kernel_spmd

    res = run_bass_kernel_spmd(
        nc, in_maps, core_ids=list(range(cfg.NCORES)), trace=trace
    )
    out = np.concatenate([r["out"] for r in res.results], axis=0)
    return out.astype(np.float32), res


def kernel(x, edge_index, W, att_src, att_dst):
    out, _ = run(DEFAULT_CFG, x, edge_index, W, att_src, att_dst)
    return out

